# revision 1
# baseline (speedup 1.0000x reference)
"""CornerProposal (eval path) Trainium2 Bass kernel.

Math: anc_centers = floor(anc_bases[..., :2]); grid_sample of 31x31 glimpses
at integer centers with align_corners=False lands every sample exactly on
half-integer pixel coords -> each output pixel is the average of a 2x2 image
neighborhood:  rois[b,n,c,i,j] = S[b,c, cy+i, cx+j]  where
S[y,x] = 0.25*(Ip[y,x]+Ip[y,x+1]+Ip[y+1,x]+Ip[y+1,x+1]) and Ip is the image
zero-padded by 16 on all sides (cx,cy = floor centers, in [0,255]).

Device strategy (per core = one batch):
 - pool the padded image into S stored row-phase-interleaved over two
   64-partition halves: partition p = 64*h + 32*q + c, row k holds
   S[y = 128*h + 2*k + q, x], x in [0,286).
 - per anchor: one 64-lane DVE copy compacts the phase-uniform 32-row
   superwindow [64p, 16, 31] into contiguous staging, then one contiguous
   DMA writes it to DRAM ([600, 2, 32, 16, 31] padded layout).
 - host de-interleaves phases and trims the superwindow to 31 rows.
"""

import sys
import threading

import numpy as np

sys.path.insert(0, "/opt/trn_rl_repo")

B, N, C, H, W = 8, 600, 32, 256, 256
K = 31
XW = 286          # stored S columns (windows reach col 285)
RK = 80           # S rows per phase per half
ICH_R, ICH_X = 33, 288
OUT_PER_ANCH = 2 * 32 * 16 * 31  # 31744

_cache = {}


def _build_program(cy, cx, reps=1):
    import concourse.bacc as bacc
    import concourse.bass as bass  # noqa: F401
    import concourse.mybir as mybir
    import concourse.tile as tile

    f32 = mybir.dt.float32
    nc = bacc.Bacc("TRN2", target_bir_lowering=False, debug=False, num_devices=1)
    img = nc.dram_tensor("img", [C, H, W], f32, kind="ExternalInput")
    rois = nc.dram_tensor("rois", [N * OUT_PER_ANCH], f32, kind="ExternalOutput")

    with tile.TileContext(nc) as tc:
        with (
            tc.tile_pool(name="main", bufs=1) as pool,
            tc.tile_pool(name="ich", bufs=2) as ipool,
        ):
            S2 = pool.tile([128, RK * XW], f32)
            S2v = S2[:].rearrange("p (k x) -> p k x", x=XW)
            stg = pool.tile([128, 8 * 496], f32)
            imga = img.ap()

            # ---------------- pooling ----------------
            for h in range(2):
                for ch in range(5):
                    rbase = 128 * h + 32 * ch          # first I_pad row in chunk
                    ic = ipool.tile([128, ICH_R * ICH_X], f32, tag="ic")
                    icv = ic[:].rearrange("p (r x) -> p r x", x=ICH_X)
                    r0v = max(rbase, 16)
                    r1v = min(rbase + ICH_R, 272)
                    l0, l1 = r0v - rbase, r1v - rbase
                    for q in range(2):
                        pb = 64 * h + 32 * q
                        # zero uncovered rows / pad columns
                        if l0 > 0:
                            nc.vector.memset(icv[pb:pb + 32, 0:l0, :], 0.0)
                        if l1 < ICH_R:
                            nc.vector.memset(icv[pb:pb + 32, l1:ICH_R, :], 0.0)
                        nc.vector.memset(icv[pb:pb + 32, l0:l1, 0:16], 0.0)
                        nc.vector.memset(icv[pb:pb + 32, l0:l1, 272:288], 0.0)
                        # load valid interior
                        nc.sync.dma_start(
                            icv[pb:pb + 32, l0:l1, 16:272],
                            imga[:, r0v - 16:r1v - 16, :],
                        )
                    for q in range(2):
                        pb = 64 * h + 32 * q
                        out = S2v[pb:pb + 32, 16 * ch:16 * ch + 16, 0:XW]
                        a = icv[pb:pb + 32, q:q + 31:2, 0:XW]
                        b_ = icv[pb:pb + 32, q:q + 31:2, 1:XW + 1]
                        c_ = icv[pb:pb + 32, q + 1:q + 32:2, 0:XW]
                        d_ = icv[pb:pb + 32, q + 1:q + 32:2, 1:XW + 1]
                        nc.vector.tensor_add(out, a, b_)
                        nc.vector.tensor_add(out, out, c_)
                        nc.gpsimd.tensor_tensor(out, out, d_, mybir.AluOpType.add)
                        nc.scalar.mul(out, out, 0.25)

            # ---------------- gather ----------------
            rf = rois.ap()
            dma_engs = [nc.sync, nc.scalar]

            def gather(_=None):
                for n in range(N):
                    s = int(cy[n])
                    c0 = int(cx[n])
                    k0 = s // 2
                    h = 0 if k0 < 64 else 1
                    kh = k0 - 64 * h
                    slot = n % 8
                    dstg = stg[64 * h:64 * h + 64, slot * 496:(slot + 1) * 496]
                    nc.vector.tensor_copy(
                        dstg, S2v[64 * h:64 * h + 64, kh:kh + 16, c0:c0 + 31]
                    )
                    dma_engs[n % 2].dma_start(
                        rf[n * OUT_PER_ANCH:(n + 1) * OUT_PER_ANCH], dstg
                    )

            if reps == 1:
                gather()
            else:
                with tc.For_i(0, reps, 1):
                    gather()

    nc.compile()
    return nc


def _centers(anc_bases):
    ctr = np.floor(anc_bases[:, :, :2].astype(np.float64))
    ctr = np.clip(ctr, 0, 255).astype(np.int64)
    return ctr[:, :, 1], ctr[:, :, 0]  # cy, cx


def _deinterleave(flat, cy):
    """flat [N*31744] device output -> [N, C, 31, 31]."""
    arr = flat.reshape(N, 2, 32, 16, 31)
    # -> [N, c, k, q, j] -> rows d = 2*k + q
    sw = arr.transpose(0, 2, 3, 1, 4).reshape(N, 32, 32, 31)
    off = (cy % 2).astype(np.int64)
    idx = off[:, None] + np.arange(31)[None, :]
    return np.take_along_axis(sw, idx[:, None, :, None], axis=2)


def kernel(images, anc_bases):
    from concourse.bass_utils import run_bass_kernel_spmd

    images = np.asarray(images, dtype=np.float32)
    anc_bases = np.asarray(anc_bases, dtype=np.float32)
    cy, cx = _centers(anc_bases)

    rois = np.empty((B, N, C, K, K), dtype=np.float32)
    for b in range(B):
        key = (cy[b].tobytes(), cx[b].tobytes())
        nc = _cache.get(key)
        if nc is None:
            nc = _build_program(cy[b], cx[b])
            _cache[key] = nc
        res = run_bass_kernel_spmd(
            nc, [{"img": np.ascontiguousarray(images[b])}], core_ids=[0]
        )
        rois[b] = _deinterleave(res.results[0]["rois"], cy[b])

    return rois, anc_bases[:, :, :2].copy()
